# revision 1
# baseline (speedup 1.0000x reference)
"""Multi-head attention (B=2, S=2048, D=1024, H=16) on 8 Trainium2 NeuronCores.

Sharding: tensor-parallel over heads — 2 heads per core. Each core computes
its heads' QKV projection, attention, and a partial FC output (row-slice of
the FC contraction); the host sums the 8 partials and adds the FC bias.

Per-core pipeline. Projection and FC matmuls run in float32r (full-rate
TF32-class); the attention loop runs in fp16 (fast weight loads, ~1e-4
relative-L2 total error). Softmax is computed without max-subtraction
(scores are bounded ~[-3, 4.5] for this problem's scale, so exp is safe
and exactly matches the reference softmax up to rounding).
  1. QKV projection: qT/kT/vT [128 feat, 4096 tok] transposed layouts;
     score scale 1/8 and biases folded into the PSUM eviction.
  2. Per (batch, head): V re-transposed to key-major [keys, 64] via the
     PE transpose, packed as [V | ones] so the AV matmul also produces
     the softmax denominators in the spare output partitions.
  3. Per key-tile: scoresT [keys, q] = K^T Q for both heads into one
     PSUM tile (concurrent PE row strips), one wide exp on ScalarE,
     then the AV accumulation trailing 2 key-tiles behind.
  4. Normalization: fast reciprocal of the denominators, partition-move
     via SBUF DMA, multiply into valuesT [128 feat, 4096 tok].
  5. FC: partial[t, e] = valuesT[:, t]^T @ w_fcT slice; DMA to DRAM.
"""
import numpy as np
from contextlib import ExitStack

import concourse.bass as bass
import concourse.tile as tile
from concourse import bacc, mybir
from concourse.bass_utils import run_bass_kernel_spmd
from concourse.masks import make_identity

B, S, D, H, HD = 2, 2048, 1024, 16, 64
T = B * S                # 4096 tokens
NC = 8                   # cores
HPC = H // NC            # heads per core
F = HPC * HD             # 128 value-features per core
KT = 128                 # key tile (contraction tile for AV)
QB = 512                 # query block (matmul free dim)
f32 = mybir.dt.float32
f32r = mybir.dt.float32r
fp16 = mybir.dt.float16
bf16 = mybir.dt.bfloat16
AF = mybir.ActivationFunctionType
OP = mybir.AluOpType

_NC_CACHE = None


def _build():
    nc = bacc.Bacc("TRN2", target_bir_lowering=False, debug=False, num_devices=NC)

    XT = nc.dram_tensor("xT", [D, T], fp16, kind="ExternalInput").ap()
    WQ = nc.dram_tensor("wq", [D, F], fp16, kind="ExternalInput").ap()
    WK = nc.dram_tensor("wk", [D, F], fp16, kind="ExternalInput").ap()
    WV = nc.dram_tensor("wv", [D, F], fp16, kind="ExternalInput").ap()
    BQ = nc.dram_tensor("bq", [F, 1], f32, kind="ExternalInput").ap()
    BK = nc.dram_tensor("bk", [F, 1], f32, kind="ExternalInput").ap()
    BV = nc.dram_tensor("bv", [F, 1], f32, kind="ExternalInput").ap()
    WFC = nc.dram_tensor("wfc", [F, D], f32r, kind="ExternalInput").ap()
    OUT = nc.dram_tensor("out", [T, D], f32, kind="ExternalOutput").ap()

    with tile.TileContext(nc) as tc, ExitStack() as ctx:
        const = ctx.enter_context(tc.tile_pool(name="const", bufs=1))
        xt_pool = ctx.enter_context(tc.tile_pool(name="xt", bufs=3))
        big = ctx.enter_context(tc.tile_pool(name="big", bufs=1))
        vt_pool = ctx.enter_context(tc.tile_pool(name="vt", bufs=4))
        exp_pool = ctx.enter_context(tc.tile_pool(name="expt", bufs=6))
        r_pool = ctx.enter_context(tc.tile_pool(name="recip", bufs=2))
        r2_pool = ctx.enter_context(tc.tile_pool(name="recip2", bufs=2))
        fout_pool = ctx.enter_context(tc.tile_pool(name="fout", bufs=6))

        ps_pool = ctx.enter_context(tc.tile_pool(name="ps_pool", bufs=1, space="PSUM"))

        # --- constants ---
        wq_sb = const.tile([128, D // 128, F], fp16)
        nc.sync.dma_start(out=wq_sb, in_=WQ.rearrange("(t p) f -> p t f", p=128))
        wk_sb = const.tile([128, D // 128, F], fp16)
        nc.sync.dma_start(out=wk_sb, in_=WK.rearrange("(t p) f -> p t f", p=128))
        wv_sb = const.tile([128, D // 128, F], fp16)
        nc.sync.dma_start(out=wv_sb, in_=WV.rearrange("(t p) f -> p t f", p=128))
        wfc_sb = const.tile([F, D], f32r)
        nc.sync.dma_start(out=wfc_sb, in_=WFC)
        bq_sb = const.tile([F, 1], f32)
        nc.sync.dma_start(out=bq_sb, in_=BQ)
        bk_sb = const.tile([F, 1], f32)
        nc.sync.dma_start(out=bk_sb, in_=BK)
        bv_sb = const.tile([F, 1], f32)
        nc.sync.dma_start(out=bv_sb, in_=BV)

        ident = const.tile([128, 64], fp16)  # I_64 stacked in both halves
        make_identity(nc, ident[0:64, :])
        make_identity(nc, ident[64:128, :])
        ones_f = const.tile([128, S // KT * HD], f32)
        nc.vector.memset(ones_f, 1.0)

        # per-token-block tiles so attention starts as soon as the blocks
        # it reads are projected (tile-granular dependency tracking)
        NTB = T // QB
        qTs = [big.tile([128, QB], fp16, name=f"qT{i}") for i in range(NTB)]
        kTs = [big.tile([128, QB], fp16, name=f"kT{i}") for i in range(NTB)]
        vTs = [big.tile([128, QB], fp16, name=f"vT{i}") for i in range(NTB)]
        valuesTs = [big.tile([128, QB], f32r, name=f"valuesT{i}")
                    for i in range(NTB)]

        # --- phase 1: QKV projection (transposed outputs) ---
        _proj_scope = nc.named_scope("proj")
        _proj_scope.__enter__()
        for tb in range(T // QB):
            xts = xt_pool.tile([128, D // 128, QB], fp16, tag="xt",
                               name=f"xts_{tb}")
            nc.sync.dma_start(
                out=xts,
                in_=XT[:, tb * QB:(tb + 1) * QB].rearrange(
                    "(kt p) t -> p kt t", p=128
                ),
            )
            for pj, (w_sb, dsts, bias_ap, scale) in enumerate((
                (wq_sb, qTs, bq_sb, 0.125),
                (wk_sb, kTs, bk_sb, None),
                (wv_sb, vTs, bv_sb, None),
            )):
                ps = ps_pool.tile([128, QB], f32, tag=f"pav{pj % 2}",
                                  name=f"proj_ps_{tb}_{pj}")
                for kt in range(D // 128):
                    nc.tensor.matmul(
                        ps, w_sb[:, kt, :], xts[:, kt, :],
                        start=(kt == 0), stop=(kt == D // 128 - 1),
                    )
                dslice = dsts[tb][:, :]
                if scale is None:
                    nc.vector.tensor_scalar_add(dslice, ps, bias_ap)
                else:
                    nc.vector.tensor_scalar(
                        dslice, ps, bias_ap, scale, op0=OP.add, op1=OP.mult
                    )

        _proj_scope.__exit__(None, None, None)

        # --- phases 2-5 per batch ---
        for b in range(B):
            t0 = b * S
            # V re-transposed to key-major + ones block for the denominators:
            # head h's lhsT tile [128 keys, 128] has V in cols hp:hp+64 (so
            # values land in psum partitions hp:hp+64) and ones in the rest.
            _vb = nc.named_scope(f"vbuild{b}")
            _vb.__enter__()
            vkm = []  # per head: [128, S//KT, 128]
            for h in range(HPC):
                hp, op_ = h * HD, (1 - h) * HD
                vk = vt_pool.tile([128, S // KT, 128], fp16, tag=f"vk{h}")
                nc.vector.tensor_copy(vk[:, :, op_:op_ + HD], ones_f)
                for kt in range(S // KT):
                    tp = ps_pool.tile([128, HD], fp16, tag=f"pav{h}",
                                      name=f"tp_{b}_{h}_{kt}")
                    nc.tensor.transpose(
                        tp,
                        vTs[b * 4 + kt // 4][h * HD:(h + 1) * HD,
                                             (kt % 4) * KT:(kt % 4 + 1) * KT],
                        ident[h * HD:(h + 1) * HD, :],
                    )
                    nc.vector.tensor_copy(vk[:, kt, hp:hp + HD], tp)
                vkm.append(vk)
            _vb.__exit__(None, None, None)

            _at = nc.named_scope(f"attn{b}")
            _at.__enter__()
            for qb in range(S // QB):
                qTq = qTs[b * 4 + qb]
                # both heads share one [128, 2*QB] score tile so exp runs as
                # a single wide ACTIVATE; the heads' score matmuls sit in
                # disjoint PE row strips and run concurrently.
                pav = [ps_pool.tile([128, QB], f32, tag=f"pav{h}",
                                    name=f"pav{h}_{b}_{qb}")
                       for h in range(HPC)]
                # AV matmuls trail the score matmuls by 2 key-tiles so the
                # exp they consume is long finished when the PE reaches them
                # (a stalled wait also blocks the PE's weight-load pull-ahead)
                AV_LAG = 3
                pending = []  # (kt, et)
                NKT = S // KT

                def emit_av(kt, et):
                    for h in range(HPC):
                        # [V|ones] lhsT: values^T into partitions hp:hp+64,
                        # softmax denominators into the other 64 partitions
                        nc.tensor.matmul(
                            pav[h], vkm[h][:, kt, :],
                            et[:, h * QB:(h + 1) * QB],
                            start=(kt == 0), stop=(kt == NKT - 1),
                        )

                for kt in range(NKT):
                    kTk = kTs[b * 4 + kt // 4]
                    k0 = (kt % 4) * KT
                    sc = ps_pool.tile([128, 2 * QB], f32, tag="sc", bufs=3)
                    for h in range(HPC):
                        hp = h * HD
                        nc.tensor.matmul(
                            sc[:, h * QB:(h + 1) * QB],
                            kTk[hp:hp + HD, k0:k0 + KT],
                            qTq[hp:hp + HD, :],
                            start=True, stop=True,
                            tile_position=(hp, 0),
                        )
                    et = exp_pool.tile([128, 2 * QB], fp16, tag="expt")
                    nc.scalar.activation(et, sc, AF.Exp)
                    pending.append((kt, et))
                    if len(pending) > AV_LAG:
                        emit_av(*pending.pop(0))
                for item in pending:
                    emit_av(*item)
                # h0: values in psum parts 0:64, denoms at 64:128 (and vice
                # versa for h1). reciprocal_approx_fast only works at base
                # partition 0, so h0 stages its denominators down via DMA
                # first; h1 recips directly and stages the result up.
                den = r_pool.tile([128, QB], f32, tag="den")
                nc.vector.tensor_copy(den[64:128, :], pav[0][64:128, :])
                den2 = r_pool.tile([64, QB], f32, tag="den2")
                nc.sync.dma_start(out=den2, in_=den[64:128, :])
                rec0 = r_pool.tile([64, QB], f32, tag="rec0")
                nc.vector.reciprocal_approx_fast(out=rec0, in_=den2)
                nc.vector.tensor_mul(
                    valuesTs[b * 4 + qb][0:64, :], pav[0][0:64, :], rec0
                )
                rec1 = r_pool.tile([64, QB], f32, tag="rec1")
                nc.vector.reciprocal_approx_fast(out=rec1, in_=pav[1][0:64, :])
                rec1b = r2_pool.tile([128, QB], f32, tag="rec1b")
                nc.sync.dma_start(out=rec1b[64:128, :], in_=rec1)
                nc.vector.tensor_mul(
                    valuesTs[b * 4 + qb][64:128, :],
                    pav[1][64:128, :],
                    rec1b[64:128, :],
                )

            _at.__exit__(None, None, None)
            # FC partial for this batch's tokens
            for tb2 in range(S // 128):
                tt = t0 + tb2 * 128
                for eb in range(D // QB):
                    fp = ps_pool.tile([128, QB], f32, tag="sc", bufs=3,
                                      name=f"fp_{b}_{tb2}_{eb}")
                    nc.tensor.matmul(
                        fp,
                        valuesTs[b * 4 + tb2 // 4][
                            :, (tb2 % 4) * 128:(tb2 % 4 + 1) * 128],
                        wfc_sb[:, eb * QB:(eb + 1) * QB],
                        start=True, stop=True,
                    )
                    fo = fout_pool.tile([128, QB], f32, tag="fout",
                                        name=f"fo_{b}_{tb2}_{eb}")
                    nc.vector.tensor_copy(fo, fp)
                    nc.sync.dma_start(
                        out=OUT[tt:tt + 128, eb * QB:(eb + 1) * QB], in_=fo
                    )

    nc.compile()
    return nc


def _get_nc():
    global _NC_CACHE
    if _NC_CACHE is None:
        _NC_CACHE = _build()
    return _NC_CACHE


def _prep_in_maps(x, w_qkv, b_qkv, w_fc):
    xT = np.ascontiguousarray(x.reshape(T, D).T).astype(np.float16)
    in_maps = []
    for c in range(NC):
        heads = [HPC * c + i for i in range(HPC)]
        rows = {
            "q": np.concatenate([np.arange(h * 3 * HD, h * 3 * HD + HD) for h in heads]),
            "k": np.concatenate([np.arange(h * 3 * HD + HD, h * 3 * HD + 2 * HD) for h in heads]),
            "v": np.concatenate([np.arange(h * 3 * HD + 2 * HD, h * 3 * HD + 3 * HD) for h in heads]),
        }
        m = {
            "xT": xT,
            "wq": np.ascontiguousarray(w_qkv[rows["q"]].T).astype(np.float16),
            "wk": np.ascontiguousarray(w_qkv[rows["k"]].T).astype(np.float16),
            "wv": np.ascontiguousarray(w_qkv[rows["v"]].T).astype(np.float16),
            "bq": np.ascontiguousarray(b_qkv[rows["q"]][:, None]),
            "bk": np.ascontiguousarray(b_qkv[rows["k"]][:, None]),
            "bv": np.ascontiguousarray(b_qkv[rows["v"]][:, None]),
            "wfc": np.ascontiguousarray(w_fc[:, c * F:(c + 1) * F].T),
        }
        in_maps.append(m)
    return in_maps


def run_kernel(inputs, trace=False, trace_cores=None):
    x = np.asarray(inputs["x"], np.float32)
    w_qkv = np.asarray(inputs["w_qkv"], np.float32)
    b_qkv = np.asarray(inputs["b_qkv"], np.float32)
    w_fc = np.asarray(inputs["w_fc"], np.float32)
    b_fc = np.asarray(inputs["b_fc"], np.float32)

    nc = _get_nc()
    in_maps = _prep_in_maps(x, w_qkv, b_qkv, w_fc)
    res = run_bass_kernel_spmd(
        nc, in_maps, core_ids=list(range(NC)), trace=trace,
        trace_cores=trace_cores,
    )
    out = res.results[0]["out"].astype(np.float32)
    for r in res.results[1:]:
        out = out + r["out"]
    out = out + b_fc[None, :]
    return out.reshape(B, S, D), res


def kernel(**inputs):
    out, _ = run_kernel(inputs, trace=False)
    return out



# revision 16
# speedup vs baseline: 1.2149x; 1.2149x over previous
"""Multi-head attention (B=2, S=2048, D=1024, H=16) on 8 Trainium2 NeuronCores.

Sharding: tensor-parallel over heads - 2 heads per core. Each core computes
its heads' QKV projection, attention, and a partial FC output (row-slice of
the FC contraction); the host sums the 8 partials and adds the FC bias.

v2: fully software-pipelined single-pass schedule. The ScalarE exp stream
(131 us of ACTIVATE) is the kernel clock; everything else (projections,
V-transposes, AV, FC, evictions, DMA) is interleaved so no engine phase
ever runs alone:
  - x is staged host-side as [tb, p, kt, t] so each token-block DMA moves
    8KB-contiguous partition lines (full DMA bandwidth; proj never waits).
  - K bias is dropped (softmax-invariant); V bias is folded into the host
    b_fc add (attention is affine in V); only the Q bias (and the 1/8 score
    scale) survives, folded into the Q eviction.
  - V reaches key-major layout via DMA xbar transposes (free engine)
    instead of PE transposes + DVE copies.
  - AV lhsT is [ones|V] / [V|ones] so the AV matmul also emits softmax
    denominators; reciprocals are broadcast across partitions by GpSimd.
  - Non-attention PE work (proj of next batch, FC of previous batch) is
    drained from a work queue between score-tile groups, keeping the PE
    warm (HAM) and the exp stream fed.
"""
import numpy as np
from collections import deque
from contextlib import ExitStack

import concourse.bass as bass
import concourse.tile as tile
from concourse import bacc, mybir
from concourse.bass_utils import run_bass_kernel_spmd

B, S, D, H, HD = 2, 2048, 1024, 16, 64
T = B * S                # 4096 tokens
NC = 8                   # cores
HPC = H // NC            # heads per core
F = HPC * HD             # 128 value-features per core
KT = 128                 # key tile
QB = 512                 # query block
NKT = S // KT            # 16 key tiles per batch
NQB = S // QB            # 4 query blocks per batch
NTB = T // QB            # 8 token blocks
f32 = mybir.dt.float32
fp16 = mybir.dt.float16
AF = mybir.ActivationFunctionType
OP = mybir.AluOpType

AV_LAG = 4

_NC_CACHE = None
_DEBUG = False


def _build():
    nc = bacc.Bacc("TRN2", target_bir_lowering=False, debug=False, num_devices=NC)

    XB = nc.dram_tensor("xb", [NTB, 128, D // 128, QB], fp16, kind="ExternalInput").ap()
    WQ = nc.dram_tensor("wq", [128, D // 128, F], fp16, kind="ExternalInput").ap()
    WK = nc.dram_tensor("wk", [128, D // 128, F], fp16, kind="ExternalInput").ap()
    WV = nc.dram_tensor("wv", [128, D // 128, F], fp16, kind="ExternalInput").ap()
    BQ = nc.dram_tensor("bq", [F, 1], f32, kind="ExternalInput").ap()
    WFC = nc.dram_tensor("wfc", [F, D], fp16, kind="ExternalInput").ap()
    OUT = nc.dram_tensor("out", [T, D], f32, kind="ExternalOutput").ap()
    if _DEBUG:
        DBG_V = nc.dram_tensor("dbg_v", [B, 128, S], fp16,
                               kind="ExternalOutput").ap()
        DBG_DEN = nc.dram_tensor("dbg_den", [8, 3, QB], f32,
                                 kind="ExternalOutput").ap()
        DBG_FOUT = nc.dram_tensor("dbg_fout", [T // 128, 128, D], f32,
                                  kind="ExternalOutput").ap()
        DBG_RB = nc.dram_tensor("dbg_rb", [4, 128, D], f32,
                                kind="ExternalOutput").ap()

    with tile.TileContext(nc) as tc, ExitStack() as ctx:
        const = ctx.enter_context(tc.tile_pool(name="const", bufs=1))
        xt_pool = ctx.enter_context(tc.tile_pool(name="xt", bufs=1))
        big = ctx.enter_context(tc.tile_pool(name="big", bufs=1))
        vk_pool = ctx.enter_context(tc.tile_pool(name="vk", bufs=1))
        et_pool = ctx.enter_context(tc.tile_pool(name="et", bufs=8))
        nrm_pool = ctx.enter_context(tc.tile_pool(name="nrm", bufs=2))
        fout_pool = ctx.enter_context(tc.tile_pool(name="fout", bufs=6))
        ps_pool = ctx.enter_context(tc.tile_pool(name="ps", bufs=1, space="PSUM"))

        # --- constants / weights ---
        wq_sb = const.tile([128, D // 128, F], fp16)
        wk_sb = const.tile([128, D // 128, F], fp16)
        wv_sb = const.tile([128, D // 128, F], fp16)
        bq_sb = const.tile([F, 1], f32)
        wfc_sb = const.tile([F, D], fp16)

        xts = [xt_pool.tile([128, D // 128, QB], fp16, name=f"xts{i}")
               for i in range(NTB)]
        qT = [big.tile([128, S], fp16, name=f"qT{b}") for b in range(B)]
        kT = [big.tile([128, S], fp16, name=f"kT{b}") for b in range(B)]
        vT = [big.tile([128, S], fp16, name=f"vT{b}") for b in range(B)]
        valuesT = [big.tile([128, S], fp16, name=f"valuesT{b}") for b in range(B)]
        # vk[b][h]: [128 keys, NKT, 128] AV lhsT. h0 = [ones | V0] (den in psum
        # parts 0:64, values 64:128); h1 = [V1 | ones] (values 0:64, den 64:128).
        vk = [[vk_pool.tile([128, NKT, 128], fp16, name=f"vk{b}_{h}")
               for h in range(HPC)] for b in range(B)]

        # DMA order: first x block 0 (unblocks K proj), then weights, rest of x.
        nc.sync.dma_start(out=xts[0], in_=XB[0])
        nc.sync.dma_start(out=wk_sb, in_=WK)
        nc.sync.dma_start(out=wq_sb, in_=WQ)
        nc.sync.dma_start(out=bq_sb, in_=BQ)
        nc.sync.dma_start(out=xts[1], in_=XB[1])
        nc.sync.dma_start(out=wv_sb, in_=WV)
        nc.sync.dma_start(out=xts[2], in_=XB[2])
        nc.sync.dma_start(out=xts[3], in_=XB[3])
        nc.sync.dma_start(out=wfc_sb, in_=WFC)
        for i in range(4, NTB):
            nc.sync.dma_start(out=xts[i], in_=XB[i])

        # warm up the ACT exp table (~2.7us load) before the first real exp
        warm = const.tile([1, 1], f32)
        nc.scalar.activation(warm, bq_sb[0:1, 0:1], AF.Exp)

        # --- helpers (issue instructions; deps handled by tile framework) ---
        def proj(kind, b, tb):
            """QKV projection of token block tb (global index) into batch b."""
            w_sb, dst = {
                "q": (wq_sb, qT[b]), "k": (wk_sb, kT[b]), "v": (wv_sb, vT[b]),
            }[kind]
            tq = (tb % NQB) * QB
            ps = ps_pool.tile([128, QB], f32, tag="pp", bufs=2,
                              name=f"pp_{kind}{tb}")
            for kt8 in range(D // 128):
                nc.tensor.matmul(ps, w_sb[:, kt8, :], xts[tb][:, kt8, :],
                                 start=(kt8 == 0), stop=(kt8 == D // 128 - 1))
            if kind == "q":
                # fold bias and the 1/8 score scale into the eviction
                nc.vector.tensor_scalar(dst[:, tq:tq + QB], ps, bq_sb, 0.125,
                                        op0=OP.add, op1=OP.mult)
            else:
                nc.vector.tensor_copy(dst[:, tq:tq + QB], ps)

        def vk_ones(b):
            # ones blocks: h0 cols 0:64, h1 cols 64:128 (per-kt contiguous
            # memsets; strided 3D memset semantics unverified on HW)
            for ktl in range(NKT):
                nc.vector.memset(vk[b][0][:, ktl, 0:HD], 1.0)
                nc.vector.memset(vk[b][1][:, ktl, HD:128], 1.0)

        def vk_trans(b, tb):
            """xbar-transpose V of token block tb (local to batch b) into vk.
            One 3D-out call per head covers 4 key tiles: out[p, g, j] =
            in[j, g*128 + p] (verified on HW)."""
            q4 = (tb % NQB) * 4
            nc.sync.dma_start_transpose(
                out=vk[b][0][:, q4:q4 + 4, HD:128],
                in_=vT[b][0:HD, (tb % NQB) * QB:(tb % NQB + 1) * QB])
            nc.sync.dma_start_transpose(
                out=vk[b][1][:, q4:q4 + 4, 0:HD],
                in_=vT[b][HD:128, (tb % NQB) * QB:(tb % NQB + 1) * QB])

        def fc_tile(b, tb2):
            """FC for one 128-token tile; both 512-wide output halves."""
            fo = fout_pool.tile([128, D], f32, tag="fout", name=f"fo_{b}_{tb2}")
            for eb in range(D // QB):
                fp = ps_pool.tile([128, QB], f32, tag="pp", bufs=2,
                                  name=f"fp_{b}_{tb2}_{eb}")
                nc.tensor.matmul(
                    fp, valuesT[b][:, tb2 * 128:(tb2 + 1) * 128],
                    wfc_sb[:, eb * QB:(eb + 1) * QB], start=True, stop=True)
                nc.vector.tensor_copy(fo[:, eb * QB:(eb + 1) * QB], fp)
            tt = b * S + tb2 * 128
            nc.sync.dma_start(out=OUT[tt:tt + 128, :], in_=fo)
            if _DEBUG:
                nc.sync.dma_start(out=DBG_FOUT[tt // 128], in_=fo)

        # --- work queue: one item drained per key-tile slot (16 per step) ---
        work = deque()
        def drain(n=1):
            for _ in range(n):
                if work:
                    work.popleft()()

        def W(fn, *a):
            work.append(lambda: fn(*a))

        # step 0 fillers: rest of batch-0 proj (K first: scores kt needs K
        # block kt//4; V feeds AV which trails by AV_LAG)
        W(proj, "k", 0, 1); W(proj, "v", 0, 1); W(vk_trans, 0, 1)
        W(proj, "q", 0, 1)
        W(proj, "k", 0, 2); W(proj, "v", 0, 2); W(vk_trans, 0, 2)
        W(proj, "q", 0, 2)
        W(proj, "k", 0, 3); W(proj, "v", 0, 3); W(vk_trans, 0, 3)
        W(proj, "q", 0, 3)
        # steps 1-2 fillers: batch-1 proj + vk
        W(vk_ones, 1)
        W(proj, "k", 1, 4); W(proj, "v", 1, 4); W(vk_trans, 1, 4)
        W(proj, "q", 1, 4)
        W(proj, "k", 1, 5); W(proj, "v", 1, 5); W(vk_trans, 1, 5)
        W(proj, "q", 1, 5)
        W(proj, "k", 1, 6); W(proj, "v", 1, 6); W(vk_trans, 1, 6)
        W(proj, "q", 1, 6)
        W(proj, "k", 1, 7); W(proj, "v", 1, 7); W(vk_trans, 1, 7)
        W(proj, "q", 1, 7)
        # NOTE: FC work is appended to the queue at the END of the step that
        # writes its valuesT slice. The tile framework derives dependencies
        # from ISSUE order -- issuing an FC matmul before the tensor_mul that
        # produces its input records no dependency at all (it would read
        # uninitialized SBUF; on warm reruns the stale data masks the race).

        # --- prefix: minimal work to start the exp stream ---
        _pre = nc.named_scope("prefix")
        _pre.__enter__()
        proj("k", 0, 0)
        proj("q", 0, 0)
        proj("v", 0, 0)
        vk_ones(0)
        vk_trans(0, 0)
        _pre.__exit__(None, None, None)

        # --- main loop: 8 attention steps pace the kernel ---
        for s in range(B * NQB):
            b, qb = divmod(s, NQB)
            _at = nc.named_scope(f"step{s}")
            _at.__enter__()
            pav = [ps_pool.tile([128, QB], f32, tag=f"pav{h}",
                                name=f"pav{h}_{s}") for h in range(HPC)]

            def av(kt, et):
                for h in range(HPC):
                    nc.tensor.matmul(pav[h], vk[b][h][:, kt, :],
                                     et[:, h * QB:(h + 1) * QB],
                                     start=(kt == 0), stop=(kt == NKT - 1))

            pending = []
            for kt in range(NKT):
                sc = ps_pool.tile([128, 2 * QB], f32, tag="sc", bufs=2,
                                  name=f"sc_{s}_{kt}")
                for h in range(HPC):
                    hp = h * HD
                    nc.tensor.matmul(
                        sc[:, h * QB:(h + 1) * QB],
                        kT[b][hp:hp + HD, kt * KT:(kt + 1) * KT],
                        qT[b][hp:hp + HD, qb * QB:(qb + 1) * QB],
                        start=True, stop=True, tile_position=(hp, 0))
                et = et_pool.tile([128, 2 * QB], fp16, tag="et")
                nc.scalar.activation(et, sc, AF.Exp)
                pending.append((kt, et))
                if len(pending) > AV_LAG:
                    av(*pending.pop(0))
                drain(1)
            for item in pending:
                av(*item)

            # normalization: recip of denominators, GpSimd partition-broadcast,
            # fused into the (PSUM -> fp16 SBUF) value eviction.
            # h0: den in pav0 parts 0:64 (all rows identical), values 64:128.
            r0 = nrm_pool.tile([128, QB], f32, tag="r0", name=f"r0_{s}")
            nc.vector.reciprocal_approx_fast(out=r0[0:1, :], in_=pav[0][0:1, :])
            # partition_broadcast only writes ranges based at partition 0 --
            # broadcast the full 128 and read the upper half (GpSimd is idle)
            r0b = nrm_pool.tile([128, QB], f32, tag="r0b", name=f"r0b_{s}")
            nc.gpsimd.partition_broadcast(r0b, r0[0:1, :])
            nc.vector.tensor_mul(valuesT[b][HD:128, qb * QB:(qb + 1) * QB],
                                 pav[0][HD:128, :], r0b[HD:128, :])
            # h1: values in parts 0:64, den in 64:128 -> row-move via DMA.
            r1 = nrm_pool.tile([128, QB], f32, tag="r1", name=f"r1_{s}")
            nc.vector.tensor_copy(r1[HD:HD + 1, :], pav[1][HD:HD + 1, :])
            r1b = nrm_pool.tile([128, QB], f32, tag="r1b", name=f"r1b_{s}")
            nc.sync.dma_start(out=r1b[0:1, :], in_=r1[HD:HD + 1, :])
            rec1 = nrm_pool.tile([128, QB], f32, tag="rec1", name=f"rec1_{s}")
            nc.vector.reciprocal_approx_fast(out=rec1[0:1, :], in_=r1b[0:1, :])
            rec1b = nrm_pool.tile([128, QB], f32, tag="rec1b", name=f"rec1b_{s}")
            nc.gpsimd.partition_broadcast(rec1b[0:HD, :], rec1[0:1, :])
            nc.vector.tensor_mul(valuesT[b][0:HD, qb * QB:(qb + 1) * QB],
                                 pav[1][0:HD, :], rec1b[0:HD, :])
            # FC for this step's tokens becomes eligible only now that its
            # valuesT inputs have been issued
            for tb2 in range(qb * 4, (qb + 1) * 4):
                W(fc_tile, b, tb2)
            if _DEBUG:
                nc.sync.dma_start(out=DBG_DEN[s, 0, :], in_=r1b[0:1, :])
                nc.sync.dma_start(out=DBG_DEN[s, 1, :], in_=r0b[64:65, :])
                nc.sync.dma_start(out=DBG_DEN[s, 2, :], in_=rec1b[0:1, :])
                nc.sync.dma_start(
                    out=DBG_V[b, :, qb * QB:(qb + 1) * QB],
                    in_=valuesT[b][:, qb * QB:(qb + 1) * QB])
            _at.__exit__(None, None, None)

        # tail: anything not yet drained (last FC tiles)
        drain(len(work))
        if _DEBUG:
            # read back the last 4 OUT tiles through SBUF to test whether
            # their DMA writes landed before kernel completion
            for i in range(4):
                tt = T - (4 - i) * 128
                rbt = fout_pool.tile([128, D], f32, tag="fout",
                                     name=f"rb_{i}")
                nc.sync.dma_start(out=rbt, in_=OUT[tt:tt + 128, :])
                nc.sync.dma_start(out=DBG_RB[i], in_=rbt)

    nc.compile()
    return nc


def _get_nc():
    global _NC_CACHE
    if _NC_CACHE is None:
        _NC_CACHE = _build()
    return _NC_CACHE


def _prep_in_maps(x, w_qkv, b_qkv, w_fc):
    # x -> [tb, p, kt, t] so each block's partition line is 8KB-contiguous
    xT = x.reshape(T, D).T.astype(np.float16)          # [D, T]
    xb = np.ascontiguousarray(
        xT.reshape(D // 128, 128, NTB, QB).transpose(2, 1, 0, 3))
    in_maps = []
    for c in range(NC):
        heads = [HPC * c + i for i in range(HPC)]
        rq = np.concatenate([np.arange(h * 3 * HD, h * 3 * HD + HD) for h in heads])
        rk = np.concatenate([np.arange(h * 3 * HD + HD, h * 3 * HD + 2 * HD) for h in heads])
        rv = np.concatenate([np.arange(h * 3 * HD + 2 * HD, h * 3 * HD + 3 * HD) for h in heads])

        def wprep(rows):
            # [D, F] -> [kt, p, F] -> SBUF tile [p, kt, F]
            wt = w_qkv[rows].T.astype(np.float16)      # [D, F]
            return np.ascontiguousarray(
                wt.reshape(D // 128, 128, F).transpose(1, 0, 2))

        # valuesT rows: 0:64 = head1 features, 64:128 = head0 features
        d1 = np.arange(heads[1] * HD, heads[1] * HD + HD)
        d0 = np.arange(heads[0] * HD, heads[0] * HD + HD)
        dperm = np.concatenate([d1, d0])
        m = {
            "xb": xb,
            "wq": wprep(rq),
            "wk": wprep(rk),
            "wv": wprep(rv),
            "bq": np.ascontiguousarray(b_qkv[rq][:, None].astype(np.float32)),
            "wfc": np.ascontiguousarray(w_fc[:, dperm].T.astype(np.float16)),
        }
        in_maps.append(m)
    return in_maps


def run_kernel(inputs, trace=False, trace_cores=None):
    x = np.asarray(inputs["x"], np.float32)
    w_qkv = np.asarray(inputs["w_qkv"], np.float32)
    b_qkv = np.asarray(inputs["b_qkv"], np.float32)
    w_fc = np.asarray(inputs["w_fc"], np.float32)
    b_fc = np.asarray(inputs["b_fc"], np.float32)

    nc = _get_nc()
    in_maps = _prep_in_maps(x, w_qkv, b_qkv, w_fc)
    res = run_bass_kernel_spmd(
        nc, in_maps, core_ids=list(range(NC)), trace=trace,
        trace_cores=trace_cores,
    )
    out = res.results[0]["out"].astype(np.float32)
    for r in res.results[1:]:
        out = out + r["out"]
    # V bias passes through attention unchanged; fold bv @ w_fc^T into the
    # host-side bias add (exact math, not an approximation).
    rv_all = np.concatenate(
        [np.arange(h * 3 * HD + 2 * HD, h * 3 * HD + 3 * HD) for h in range(H)])
    bv_full = b_qkv[rv_all]
    out = out + b_fc[None, :] + (w_fc @ bv_full)[None, :]
    return out.reshape(B, S, D), res


def kernel(**inputs):
    out, _ = run_kernel(inputs, trace=False)
    return out


# revision 32
# speedup vs baseline: 1.3183x; 1.0851x over previous
"""Multi-head attention (B=2, S=2048, D=1024, H=16) on 8 Trainium2 NeuronCores.

Sharding: tensor-parallel over heads - 2 heads per core. Each core computes
its heads' QKV projection, attention, and a partial FC output (row-slice of
the FC contraction); the host sums the 8 partials and adds the FC bias.

v2: fully software-pipelined single-pass schedule. The ScalarE exp stream
(131 us of ACTIVATE) is the kernel clock; everything else (projections,
V-transposes, AV, FC, evictions, DMA) is interleaved so no engine phase
ever runs alone:
  - x is staged host-side as [tb, p, kt, t] so each token-block DMA moves
    8KB-contiguous partition lines (full DMA bandwidth; proj never waits).
  - K bias is dropped (softmax-invariant); V bias is folded into the host
    b_fc add (attention is affine in V); only the Q bias (and the 1/8 score
    scale) survives, folded into the Q eviction.
  - V reaches key-major layout via DMA xbar transposes (free engine)
    instead of PE transposes + DVE copies.
  - AV lhsT is [ones|V] / [V|ones] so the AV matmul also emits softmax
    denominators; reciprocals are broadcast across partitions by GpSimd.
  - Non-attention PE work (proj of next batch, FC of previous batch) is
    drained from a work queue between score-tile groups, keeping the PE
    warm (HAM) and the exp stream fed.
"""
import numpy as np
from collections import deque
from contextlib import ExitStack

import concourse.bass as bass
import concourse.tile as tile
from concourse import bacc, mybir
from concourse.bass_utils import run_bass_kernel_spmd

B, S, D, H, HD = 2, 2048, 1024, 16, 64
T = B * S                # 4096 tokens
NC = 8                   # cores
HPC = H // NC            # heads per core
F = HPC * HD             # 128 value-features per core
KT = 128                 # key tile
QB = 512                 # query block
NKT = S // KT            # 16 key tiles per batch
NQB = S // QB            # 4 query blocks per batch
NTB = T // QB            # 8 token blocks
f32 = mybir.dt.float32
fp16 = mybir.dt.float16
AF = mybir.ActivationFunctionType
OP = mybir.AluOpType

AV_LAG = 4

_NC_CACHE = None
_DEBUG = False


def _build():
    nc = bacc.Bacc("TRN2", target_bir_lowering=False, debug=False, num_devices=NC)

    XB = nc.dram_tensor("xb", [NTB, 128, D // 128, QB], fp16, kind="ExternalInput").ap()
    WQ = nc.dram_tensor("wq", [128, D // 128, F], fp16, kind="ExternalInput").ap()
    WK = nc.dram_tensor("wk", [128, D // 128, F], fp16, kind="ExternalInput").ap()
    WV = nc.dram_tensor("wv", [128, D // 128, F], fp16, kind="ExternalInput").ap()
    BQ = nc.dram_tensor("bq", [F, 1], f32, kind="ExternalInput").ap()
    WFC = nc.dram_tensor("wfc", [F, D], fp16, kind="ExternalInput").ap()
    OUT = nc.dram_tensor("out", [T, D], f32, kind="ExternalOutput").ap()
    if _DEBUG:
        DBG_V = nc.dram_tensor("dbg_v", [B, 128, S], fp16,
                               kind="ExternalOutput").ap()
        DBG_DEN = nc.dram_tensor("dbg_den", [8, 3, QB], f32,
                                 kind="ExternalOutput").ap()
        DBG_FOUT = nc.dram_tensor("dbg_fout", [T // 128, 128, D], f32,
                                  kind="ExternalOutput").ap()
        DBG_RB = nc.dram_tensor("dbg_rb", [4, 128, D], f32,
                                kind="ExternalOutput").ap()
        DBG_VK = nc.dram_tensor("dbg_vk", [2, 128, NKT, 128], fp16,
                                kind="ExternalOutput").ap()
        DBG_KQV = nc.dram_tensor("dbg_kqv", [3, 128, S], fp16,
                                 kind="ExternalOutput").ap()
        DBG_ET = nc.dram_tensor("dbg_et", [NKT, 128, 2 * QB], fp16,
                                kind="ExternalOutput").ap()
        DBG_PAV = nc.dram_tensor("dbg_pav", [2, 128, QB], f32,
                                 kind="ExternalOutput").ap()

    with tile.TileContext(nc) as tc, ExitStack() as ctx:
        const = ctx.enter_context(tc.tile_pool(name="const", bufs=1))
        xt_pool = ctx.enter_context(tc.tile_pool(name="xt", bufs=1))
        big = ctx.enter_context(tc.tile_pool(name="big", bufs=1))
        vk_pool = ctx.enter_context(tc.tile_pool(name="vk", bufs=1))
        et_pool = ctx.enter_context(tc.tile_pool(name="et", bufs=8))
        nrm_pool = ctx.enter_context(tc.tile_pool(name="nrm", bufs=2))
        fout_pool = ctx.enter_context(tc.tile_pool(name="fout", bufs=6))
        ps_pool = ctx.enter_context(tc.tile_pool(name="ps", bufs=1, space="PSUM"))

        # --- constants / weights ---
        wq_sb = const.tile([128, D // 128, F], fp16)
        wk_sb = const.tile([128, D // 128, F], fp16)
        wv_sb = const.tile([128, D // 128, F], fp16)
        bq_sb = const.tile([F, 1], f32)
        wfc_sb = const.tile([F, D], fp16)

        xts = [xt_pool.tile([128, D // 128, QB], fp16, name=f"xts{i}")
               for i in range(NTB)]
        qT = [big.tile([128, S], fp16, name=f"qT{b}") for b in range(B)]
        kT = [big.tile([128, S], fp16, name=f"kT{b}") for b in range(B)]
        vT = [big.tile([128, S], fp16, name=f"vT{b}") for b in range(B)]
        valuesT = [big.tile([128, S], fp16, name=f"valuesT{b}") for b in range(B)]
        # vk[b][h]: [128 keys, NKT, 128] AV lhsT. h0 = [ones | V0] (den in psum
        # parts 0:64, values 64:128); h1 = [V1 | ones] (values 0:64, den 64:128).
        vk = [[vk_pool.tile([128, NKT, 128], fp16, name=f"vk{b}_{h}")
               for h in range(HPC)] for b in range(B)]

        # DMA order: first x block 0 (unblocks K proj), then weights, rest of x.
        nc.sync.dma_start(out=xts[0], in_=XB[0])
        nc.sync.dma_start(out=wk_sb, in_=WK)
        nc.sync.dma_start(out=wq_sb, in_=WQ)
        nc.sync.dma_start(out=bq_sb, in_=BQ)
        nc.sync.dma_start(out=xts[1], in_=XB[1])
        nc.sync.dma_start(out=wv_sb, in_=WV)
        nc.sync.dma_start(out=xts[2], in_=XB[2])
        nc.sync.dma_start(out=xts[3], in_=XB[3])
        nc.sync.dma_start(out=wfc_sb, in_=WFC)
        for i in range(4, NTB):
            nc.sync.dma_start(out=xts[i], in_=XB[i])

        # warm up the ACT exp table (~2.7us load) before the first real exp
        warm = const.tile([1, 1], f32)
        nc.scalar.activation(warm, bq_sb[0:1, 0:1], AF.Exp)

        # --- helpers (issue instructions; deps handled by tile framework) ---
        proj_ps = {}

        def proj_half(kind, b, tb, half):
            """Half a QKV projection group (4 of 8 contraction matmuls).
            Split so drained PE work injects <=1us bubbles into the
            score->exp stream."""
            w_sb, dst = {
                "q": (wq_sb, qT[b]), "k": (wk_sb, kT[b]), "v": (wv_sb, vT[b]),
            }[kind]
            tq = (tb % NQB) * QB
            if half == 0:
                ps = ps_pool.tile([128, QB], f32, tag="pp", bufs=2,
                                  name=f"pp_{kind}{tb}")
                proj_ps[(kind, tb)] = ps
            else:
                ps = proj_ps.pop((kind, tb))
            for kt8 in range(half * 4, half * 4 + 4):
                nc.tensor.matmul(ps, w_sb[:, kt8, :], xts[tb][:, kt8, :],
                                 start=(kt8 == 0), stop=(kt8 == D // 128 - 1))
            if half == 1:
                if kind == "q":
                    # fold bias and the 1/8 score scale into the eviction
                    nc.vector.tensor_scalar(dst[:, tq:tq + QB], ps, bq_sb,
                                            0.125, op0=OP.add, op1=OP.mult)
                else:
                    nc.vector.tensor_copy(dst[:, tq:tq + QB], ps)

        def proj(kind, b, tb):
            proj_half(kind, b, tb, 0)
            proj_half(kind, b, tb, 1)

        def vk_ones(b, half=None):
            # ones blocks: h0 cols 0:64, h1 cols 64:128 (per-kt contiguous
            # memsets; strided 3D memset semantics unverified on HW)
            rng = range(NKT) if half is None else \
                range(half * NKT // 2, (half + 1) * NKT // 2)
            for ktl in rng:
                nc.vector.memset(vk[b][0][:, ktl, 0:HD], 1.0)
                nc.vector.memset(vk[b][1][:, ktl, HD:128], 1.0)

        def vk_trans(b, tb):
            """xbar-transpose V of token block tb (local to batch b) into vk.
            One 3D-out call per head covers 4 key tiles: out[p, g, j] =
            in[j, g*128 + p]. The xbar writes a CONTIGUOUS staging tile
            (non-contiguous transpose destinations produce wrong output on
            HW -- see tile_matmul.py); a DVE copy then scatters into vk."""
            q4 = (tb % NQB) * 4
            for h, vcol in ((0, HD), (1, 0)):
                stg = nrm_pool.tile([128, 4, HD], fp16, tag="vstg", bufs=4,
                                    name=f"stg_{b}_{tb}_{h}")
                nc.sync.dma_start_transpose(
                    out=stg,
                    in_=vT[b][h * HD:(h + 1) * HD,
                              (tb % NQB) * QB:(tb % NQB + 1) * QB])
                nc.vector.tensor_copy(
                    vk[b][h][:, q4:q4 + 4, vcol:vcol + HD], stg)

        def fc_half(b, tb2, eb, evict_engine="v"):
            """FC for one 128-token tile, one 512-wide output half."""
            fp = ps_pool.tile([128, QB], f32, tag="pp", bufs=2,
                              name=f"fp_{b}_{tb2}_{eb}")
            nc.tensor.matmul(
                fp, valuesT[b][:, tb2 * 128:(tb2 + 1) * 128],
                wfc_sb[:, eb * QB:(eb + 1) * QB], start=True, stop=True)
            fo = fout_pool.tile([128, QB], f32, tag="fout",
                                name=f"fo_{b}_{tb2}_{eb}")
            if evict_engine == "s":
                nc.scalar.copy(fo, fp)
            else:
                nc.vector.tensor_copy(fo, fp)
            tt = b * S + tb2 * 128
            nc.sync.dma_start(out=OUT[tt:tt + 128, eb * QB:(eb + 1) * QB],
                              in_=fo)
            if _DEBUG:
                nc.sync.dma_start(out=DBG_FOUT[tt // 128, :, eb * QB:
                                               (eb + 1) * QB], in_=fo)

        # --- work queue: one item drained per key-tile slot (16 per step) ---
        work = deque()
        def drain(n=1):
            for _ in range(n):
                if work:
                    work.popleft()()

        def W(fn, *a):
            work.append(lambda: fn(*a))

        # NOTE on ordering: the tile framework derives dependencies from
        # ISSUE order -- issuing an instruction before the one producing its
        # input records no dependency at all (it reads uninitialized SBUF;
        # on warm reruns the stale data happens to equal the correct data,
        # masking the race). So every work item must be drained AFTER its
        # producers are issued: Q/K/V/T of batch 1 before step 4, FC of
        # (b, qb) after step (b, qb)'s normalization.

        def PH(kind, b, tb):
            W(proj_half, kind, b, tb, 0)
            W(proj_half, kind, b, tb, 1)

        # per-step static fillers, pushed at step start
        sched = [[] for _ in range(8)]

        def push_step(s):
            for item in sched[s]:
                work.append(item)

        def L(s, fn, *a):
            sched[s].append(lambda: fn(*a))

        def LPH(s, kind, b, tb):
            L(s, proj_half, kind, b, tb, 0)
            L(s, proj_half, kind, b, tb, 1)

        # step 0: rest of batch-0 proj. HARD ISSUE DEADLINES (see NOTE):
        # K block tb before score iteration 4*tb, vk transpose tb before the
        # trailing AV(4*tb) is issued, Q(qb) before step qb. All K's first,
        # then V+transpose pairs, then Q's; step 0 drains 2 items per slot.
        for tb in (1, 2, 3):
            LPH(0, "k", 0, tb)
        for tb in (1, 2, 3):
            LPH(0, "v", 0, tb)
            L(0, vk_trans, 0, tb)
        for tb in (1, 2, 3):
            LPH(0, "q", 0, tb)
        # steps 1-3: batch-1 proj + vk (all needed before step 4)
        L(1, vk_ones, 1, 0)
        L(1, vk_ones, 1, 1)
        LPH(1, "k", 1, 4); LPH(1, "v", 1, 4); L(1, vk_trans, 1, 4)
        LPH(2, "k", 1, 5); LPH(2, "v", 1, 5); L(2, vk_trans, 1, 5)
        LPH(3, "q", 1, 4)
        LPH(3, "k", 1, 6); LPH(3, "v", 1, 6); L(3, vk_trans, 1, 6)
        LPH(3, "k", 1, 7); LPH(3, "v", 1, 7); L(3, vk_trans, 1, 7)
        LPH(4, "q", 1, 5)
        LPH(5, "q", 1, 6)
        LPH(6, "q", 1, 7)
        # FC is deferred into the ACT-paced batch-1 phase (PE has slack there)
        fc_sched = {4: [(0, 0), (0, 1)], 5: [(0, 2), (0, 3)],
                    6: [(1, 0), (1, 1)], 7: [(1, 2)]}
        for s5, qbs in fc_sched.items():
            for fb, fqb in qbs:
                for tb2 in range(fqb * 4, (fqb + 1) * 4):
                    for eb in range(D // QB):
                        L(s5, fc_half, fb, tb2, eb)

        # --- prefix: minimal work to start the exp stream ---
        _pre = nc.named_scope("prefix")
        _pre.__enter__()
        proj("k", 0, 0)
        proj("q", 0, 0)
        proj("v", 0, 0)
        vk_ones(0)
        vk_trans(0, 0)
        _pre.__exit__(None, None, None)

        # --- main loop: 8 attention steps pace the kernel ---
        for s in range(B * NQB):
            b, qb = divmod(s, NQB)
            push_step(s)
            _at = nc.named_scope(f"step{s}")
            _at.__enter__()
            pav = [ps_pool.tile([128, QB], f32, tag=f"pav{h}",
                                name=f"pav{h}_{s}") for h in range(HPC)]

            def av(kt, et):
                for h in range(HPC):
                    nc.tensor.matmul(pav[h], vk[b][h][:, kt, :],
                                     et[:, h * QB:(h + 1) * QB],
                                     start=(kt == 0), stop=(kt == NKT - 1))

            pending = []
            for kt in range(NKT):
                sc = ps_pool.tile([128, 2 * QB], f32, tag="sc", bufs=2,
                                  name=f"sc_{s}_{kt}")
                for h in range(HPC):
                    hp = h * HD
                    nc.tensor.matmul(
                        sc[:, h * QB:(h + 1) * QB],
                        kT[b][hp:hp + HD, kt * KT:(kt + 1) * KT],
                        qT[b][hp:hp + HD, qb * QB:(qb + 1) * QB],
                        start=True, stop=True, tile_position=(hp, 0))
                if s >= 4 and kt in (6, 14):
                    # Schraudolph exp on the DVE: exp(x) ~= bitcast_fp16(
                    # round(x*1477.32 + 15360)). Softmax cancels the bias;
                    # residual jitter ~1.8% on 2/16 of the keys. Offloads
                    # the ACT-bound batch-1 phase.
                    eti = et_pool.tile([128, 2 * QB], mybir.dt.int16,
                                       tag="eti", bufs=3)
                    nc.vector.tensor_scalar(eti, sc, 1477.3197, 15360.0,
                                            op0=OP.mult, op1=OP.add)
                    et = eti[:, :].bitcast(fp16)
                else:
                    et = et_pool.tile([128, 2 * QB], fp16, tag="et")
                    nc.scalar.activation(et, sc, AF.Exp)
                if _DEBUG and s == 0:
                    nc.sync.dma_start(out=DBG_ET[kt], in_=et)
                pending.append((kt, et))
                if len(pending) > AV_LAG:
                    av(*pending.pop(0))
                drain(2 if s == 0 else 1)
            for item in pending:
                av(*item)
            if _DEBUG and s == 0:
                for h in range(HPC):
                    pc = nrm_pool.tile([128, QB], f32, tag="dbgpav", bufs=2,
                                       name=f"dbgpav{h}")
                    nc.vector.tensor_copy(pc, pav[h])
                    nc.sync.dma_start(out=DBG_PAV[h], in_=pc)

            # normalization: recip of denominators, GpSimd partition-broadcast,
            # fused into the (PSUM -> fp16 SBUF) value eviction.
            # h0: den in pav0 parts 0:64 (all rows identical), values 64:128.
            r0 = nrm_pool.tile([128, QB], f32, tag="r0", name=f"r0_{s}")
            nc.vector.reciprocal_approx_fast(out=r0[0:1, :], in_=pav[0][0:1, :])
            # partition_broadcast only writes ranges based at partition 0 --
            # broadcast the full 128 and read the upper half (GpSimd is idle)
            r0b = nrm_pool.tile([128, QB], f32, tag="r0b", name=f"r0b_{s}")
            nc.gpsimd.partition_broadcast(r0b, r0[0:1, :])
            nc.vector.tensor_mul(valuesT[b][HD:128, qb * QB:(qb + 1) * QB],
                                 pav[0][HD:128, :], r0b[HD:128, :])
            # h1: values in parts 0:64, den in 64:128 -> row-move via DMA.
            r1 = nrm_pool.tile([128, QB], f32, tag="r1", name=f"r1_{s}")
            nc.vector.tensor_copy(r1[HD:HD + 1, :], pav[1][HD:HD + 1, :])
            r1b = nrm_pool.tile([128, QB], f32, tag="r1b", name=f"r1b_{s}")
            nc.sync.dma_start(out=r1b[0:1, :], in_=r1[HD:HD + 1, :])
            rec1 = nrm_pool.tile([128, QB], f32, tag="rec1", name=f"rec1_{s}")
            nc.vector.reciprocal_approx_fast(out=rec1[0:1, :], in_=r1b[0:1, :])
            rec1b = nrm_pool.tile([128, QB], f32, tag="rec1b", name=f"rec1b_{s}")
            nc.gpsimd.partition_broadcast(rec1b[0:HD, :], rec1[0:1, :])
            nc.vector.tensor_mul(valuesT[b][0:HD, qb * QB:(qb + 1) * QB],
                                 pav[1][0:HD, :], rec1b[0:HD, :])
            if _DEBUG and s == 0:
                for h in range(HPC):
                    nc.sync.dma_start(out=DBG_VK[h], in_=vk[0][h])
                nc.sync.dma_start(out=DBG_KQV[0], in_=kT[0])
                nc.sync.dma_start(out=DBG_KQV[1], in_=qT[0])
                nc.sync.dma_start(out=DBG_KQV[2], in_=vT[0])
            if _DEBUG:
                nc.sync.dma_start(out=DBG_DEN[s, 0, :], in_=r1b[0:1, :])
                nc.sync.dma_start(out=DBG_DEN[s, 1, :], in_=r0b[64:65, :])
                nc.sync.dma_start(out=DBG_DEN[s, 2, :], in_=rec1b[0:1, :])
                nc.sync.dma_start(
                    out=DBG_V[b, :, qb * QB:(qb + 1) * QB],
                    in_=valuesT[b][:, qb * QB:(qb + 1) * QB])
            _at.__exit__(None, None, None)

        # tail: leftover queue items, then the last FC block
        drain(len(work))
        for tb2 in range(12, 16):
            for eb in range(D // QB):
                fc_half(1, tb2, eb)

    nc.compile()
    return nc


def _get_nc():
    global _NC_CACHE
    if _NC_CACHE is None:
        _NC_CACHE = _build()
    return _NC_CACHE


def _prep_in_maps(x, w_qkv, b_qkv, w_fc):
    # x -> [tb, p, kt, t] so each block's partition line is 8KB-contiguous
    xT = x.reshape(T, D).T.astype(np.float16)          # [D, T]
    xb = np.ascontiguousarray(
        xT.reshape(D // 128, 128, NTB, QB).transpose(2, 1, 0, 3))
    in_maps = []
    for c in range(NC):
        heads = [HPC * c + i for i in range(HPC)]
        rq = np.concatenate([np.arange(h * 3 * HD, h * 3 * HD + HD) for h in heads])
        rk = np.concatenate([np.arange(h * 3 * HD + HD, h * 3 * HD + 2 * HD) for h in heads])
        rv = np.concatenate([np.arange(h * 3 * HD + 2 * HD, h * 3 * HD + 3 * HD) for h in heads])

        def wprep(rows):
            # [D, F] -> [kt, p, F] -> SBUF tile [p, kt, F]
            wt = w_qkv[rows].T.astype(np.float16)      # [D, F]
            return np.ascontiguousarray(
                wt.reshape(D // 128, 128, F).transpose(1, 0, 2))

        # valuesT rows: 0:64 = head1 features, 64:128 = head0 features
        d1 = np.arange(heads[1] * HD, heads[1] * HD + HD)
        d0 = np.arange(heads[0] * HD, heads[0] * HD + HD)
        dperm = np.concatenate([d1, d0])
        m = {
            "xb": xb,
            "wq": wprep(rq),
            "wk": wprep(rk),
            "wv": wprep(rv),
            "bq": np.ascontiguousarray(b_qkv[rq][:, None].astype(np.float32)),
            "wfc": np.ascontiguousarray(w_fc[:, dperm].T.astype(np.float16)),
        }
        in_maps.append(m)
    return in_maps


def run_kernel(inputs, trace=False, trace_cores=None):
    x = np.asarray(inputs["x"], np.float32)
    w_qkv = np.asarray(inputs["w_qkv"], np.float32)
    b_qkv = np.asarray(inputs["b_qkv"], np.float32)
    w_fc = np.asarray(inputs["w_fc"], np.float32)
    b_fc = np.asarray(inputs["b_fc"], np.float32)

    nc = _get_nc()
    in_maps = _prep_in_maps(x, w_qkv, b_qkv, w_fc)
    res = run_bass_kernel_spmd(
        nc, in_maps, core_ids=list(range(NC)), trace=trace,
        trace_cores=trace_cores,
    )
    out = res.results[0]["out"].astype(np.float32)
    for r in res.results[1:]:
        out = out + r["out"]
    # V bias passes through attention unchanged; fold bv @ w_fc^T into the
    # host-side bias add (exact math, not an approximation).
    rv_all = np.concatenate(
        [np.arange(h * 3 * HD + 2 * HD, h * 3 * HD + 3 * HD) for h in range(H)])
    bv_full = b_qkv[rv_all]
    out = out + b_fc[None, :] + (w_fc @ bv_full)[None, :]
    return out.reshape(B, S, D), res


def kernel(**inputs):
    out, _ = run_kernel(inputs, trace=False)
    return out
